# revision 1
# baseline (speedup 1.0000x reference)
"""v2: interleaved transformer block kernel for TRN2 (8 cores).

Same sharding as kernel.py. Differences:
- K^T/V/Q^T/E^T stored bf16; K/V projections consume bf16 operands
  (host-provided bf16 copies of wk/wv/xfT/efT) -> less DMA, less SBUF
- within each attention block, the half-1 K/V projection matmuls are
  interleaved into the ACT-bound half-0 attention loop, and the first half
  of the Wo contraction is interleaved into the half-1 attention loop
- softmax, Wo, MLP, LN stay f32r/f32
"""

import os
import sys

import numpy as np
import ml_dtypes

for _p in ("/opt/trn_rl_repo", "/root/.axon_site/_ro/trn_rl_repo"):
    if os.path.isdir(_p) and _p not in sys.path:
        sys.path.append(_p)

import concourse.bass as bass
import concourse.mybir as mybir
import concourse.tile as tile
from concourse import bacc
from concourse.bass_utils import run_bass_kernel_spmd
from concourse.masks import make_identity

B, S, D = 2, 2048, 1024
H, DH = 16, 64
INNER = H * DH
FF = 4096
SCALE = DH ** -0.5
EPS = 1e-5
QS = 512
P = 128
NEG = -1e9
NCORES = 8

F32 = mybir.dt.float32
F32R = mybir.dt.float32r
BF = mybir.dt.bfloat16
AF = mybir.ActivationFunctionType
ALU = mybir.AluOpType

_SENT = object()


def drain(gen):
    for _ in gen:
        pass


def interleave(primary, filler, rate):
    acc = 0.0
    for _ in primary:
        if filler is None:
            continue
        acc += rate
        while acc >= 1.0:
            acc -= 1.0
            if next(filler, _SENT) is _SENT:
                filler = None
                break
    if filler is not None:
        drain(filler)


def _dram_bcast(vec_ap, parts):
    return bass.AP(tensor=vec_ap.tensor, offset=vec_ap.offset,
                   ap=[[0, parts]] + list(vec_ap.ap))


def _layernorm_q(nc, pool, name, h_sb, q, g_sb, b_sb, dst, eps_sb):
    stats = pool.tile([P, 2, 6], F32, tag="lnst", bufs=2, name=f"{name}_st")
    for sub in range(2):
        nc.vector.bn_stats(out=stats[:, sub, :],
                           in_=h_sb[:, q, sub * 512:(sub + 1) * 512])
    mv = pool.tile([P, 2], F32, tag="lnmv", bufs=2, name=f"{name}_mv")
    nc.vector.bn_aggr(out=mv, in_=stats)
    std = pool.tile([P, 1], F32, tag="lnstd", bufs=2, name=f"{name}_sd")
    nc.scalar.activation(out=std, in_=mv[:, 1:2], func=AF.Sqrt, bias=eps_sb)
    rstd = pool.tile([P, 1], F32, tag="lnrs", bufs=2, name=f"{name}_rs")
    nc.vector.reciprocal(rstd, std)
    nc.vector.tensor_scalar(out=dst, in0=h_sb[:, q, :], scalar1=mv[:, 0:1],
                            scalar2=rstd, op0=ALU.subtract, op1=ALU.mult)
    nc.gpsimd.tensor_tensor(out=dst, in0=dst, in1=g_sb, op=ALU.mult)
    nc.gpsimd.tensor_tensor(out=dst, in0=dst, in1=b_sb, op=ALU.add)


def gen_qproj(nc, qt_sb, w_dram, rhs_fn, wtag, pool, psp):
    """qt_sb[:, m, :] (bf16) = sum_d w[d, m].T @ rhs(d) (f32r operands)."""
    w_re = w_dram.rearrange("(c p) n -> p c n", p=P)
    for m in range(8):
        w_m = pool.tile([P, 8, P], F32R, tag="qp_w", bufs=3, name=f"qpw_{wtag}{m}")
        nc.sync.dma_start(out=w_m, in_=w_re[:, :, m * P:(m + 1) * P])
        ps = psp.tile([P, QS], F32, tag="qp_ps", bufs=2, name=f"qpps_{wtag}{m}")
        for d in range(8):
            nc.tensor.matmul(ps, w_m[:, d, :], rhs_fn(d),
                             start=(d == 0), stop=(d == 7))
        nc.vector.tensor_copy(out=qt_sb[:, m, :], in_=ps)
        yield


def gen_kvproj(nc, name, hf, kt_sb, v_sb, srcT_dram, wk_dram, wv_dram,
               kvp, kvps, ones_f32):
    """K^T/V projection for half hf from bf16 operands. ~2 matmuls/quantum.
    kt_sb [128, 4, 2048] bf16; v_sb [128, 16, 8, 65] bf16."""
    NSL = 512
    wk_re = wk_dram.rearrange("(c p) n -> p c n", p=P)
    wv_re = wv_dram.rearrange("(c p) n -> p c n", p=P)
    srcT_re = srcT_dram.rearrange("(c p) s -> p c s", p=P)
    hi = hf * 512
    wk_h = kvp.tile([P, 8, 512], BF, tag="wk", name=f"{name}_wk{hf}")
    wv_h = kvp.tile([P, 8, 512], BF, tag="wv", name=f"{name}_wv{hf}")
    for d in range(8):
        nc.sync.dma_start(out=wk_h[:, d, :], in_=wk_re[:, d, hi:hi + 512])
        nc.sync.dma_start(out=wv_h[:, d, :], in_=wv_re[:, d, hi:hi + 512])
    for s in range(S // NSL):
        xf_s = kvp.tile([P, 8, NSL], BF, tag="xf", bufs=2,
                        name=f"{name}_xf{hf}_{s}")
        for d in range(8):
            nc.sync.dma_start(out=xf_s[:, d, :],
                              in_=srcT_re[:, d, s * NSL:(s + 1) * NSL])
        for m in range(4):
            ps = kvps.tile([P, NSL], F32, tag="kv_ps", bufs=2,
                           name=f"{name}_psk{hf}_{s}_{m}")
            for d in range(8):
                nc.tensor.matmul(ps, wk_h[:, d, m * P:(m + 1) * P],
                                 xf_s[:, d, :], start=(d == 0), stop=(d == 7))
                if d % 2 == 1:
                    yield
            nc.vector.tensor_copy(out=kt_sb[:, m, s * NSL:(s + 1) * NSL],
                                  in_=ps)
        for c in range(NSL // P):
            ps = kvps.tile([P, 512], F32, tag="kv_ps", bufs=2,
                           name=f"{name}_psv{hf}_{s}_{c}")
            for d in range(8):
                nc.tensor.matmul(ps, xf_s[:, d, c * P:(c + 1) * P],
                                 wv_h[:, d, :], start=(d == 0), stop=(d == 7))
                if d % 2 == 1:
                    yield
            nc.vector.tensor_copy(
                out=v_sb[:, s * (NSL // P) + c, :, 0:64],
                in_=ps.rearrange("p (h e) -> p h e", h=8))
    nc.vector.tensor_copy(
        out=v_sb[:, :, :, 64:65],
        in_=ones_f32[:, 0:128].rearrange("p (a b c) -> p a b c", a=16, b=8))
    yield


def gen_attn(nc, name, hf, kt_sb, v_sb, qt_sb, ot_sb, mask_sb, ones_sb,
             misc, atps):
    """One quantum per (h, j) step; AV lags one step; divide chains inline."""
    psum_o = {}
    prev = None
    pending_div = None

    def divide_chain(h_loc):
        h_glob = hf * 8 + h_loc
        g = h_glob // 2
        poff = (h_glob % 2) * 64
        po = psum_o.pop(h_loc)
        rcp = misc.tile([1, QS], F32R, tag="rcp", bufs=2,
                        name=f"{name}_rcp{hf}_{h_loc}")
        with nc.allow_low_precision(reason="f32r softmax denom"):
            nc.vector.reciprocal(rcp, po[64:65, :])
        psb = atps.tile([64, QS], F32, tag="psb", bufs=1,
                        name=f"{name}_psb{hf}_{h_loc}")
        nc.tensor.matmul(psb, ones_sb[0:1, :], rcp, start=True, stop=True)
        rb = misc.tile([64, QS], F32, tag="rb", bufs=2,
                       name=f"{name}_rb{hf}_{h_loc}")
        nc.vector.tensor_copy(out=rb, in_=psb)
        if poff == 0:
            nc.vector.tensor_tensor(out=ot_sb[0:64, g, :], in0=po[0:64, :],
                                    in1=rb, op=ALU.mult)
        else:
            tmp = misc.tile([64, QS], F32R, tag="tdiv", bufs=2,
                            name=f"{name}_td{hf}_{h_loc}")
            nc.vector.tensor_tensor(out=tmp, in0=po[0:64, :], in1=rb,
                                    op=ALU.mult)
            nc.sync.dma_start(out=ot_sb[64:128, g, :], in_=tmp)

    for h_loc in range(8):
        h_glob = hf * 8 + h_loc
        m_loc = h_loc // 2
        poff = (h_loc % 2) * 64
        g = h_glob // 2
        psum_o[h_loc] = atps.tile([65, QS], F32, tag="pso", bufs=2,
                                  name=f"{name}_pso{hf}_{h_loc}")
        for j in range(16):
            ps_s = atps.tile([P, QS], F32, tag="pss", bufs=3,
                             name=f"{name}_pss{hf}_{h_loc}_{j}")
            nc.tensor.matmul(ps_s,
                             kt_sb[poff:poff + 64, m_loc, j * P:(j + 1) * P],
                             qt_sb[poff:poff + 64, g, :],
                             start=True, stop=True, tile_position=(poff, 0))
            et = misc.tile([P, QS], BF, tag="et", bufs=4,
                           name=f"{name}_et{hf}_{h_loc}_{j}")
            nc.scalar.activation(out=et, in_=ps_s, func=AF.Exp,
                                 bias=mask_sb[:, j:j + 1], scale=1.0)
            if prev is not None:
                ph, pj, pet = prev
                nc.tensor.matmul(psum_o[ph], v_sb[:, pj, ph, :], pet,
                                 start=(pj == 0), stop=(pj == 15))
            prev = (h_loc, j, et)
            if pending_div is not None and j == 4:
                divide_chain(pending_div)
                pending_div = None
            yield
        if h_loc == 7:
            ph, pj, pet = prev
            nc.tensor.matmul(psum_o[ph], v_sb[:, pj, ph, :], pet,
                             start=(pj == 0), stop=(pj == 15))
            prev = None
            if pending_div is not None:
                divide_chain(pending_div)
            divide_chain(7)
        else:
            pending_div = h_loc


def gen_wo_partial(nc, name, ot_sb, wo_dram, partial_sb, wop, wops):
    """Wo contraction over inner chunks 0-3 -> partial_sb [128, 4, 1024] f32."""
    wo_re = wo_dram.rearrange("(c p) n -> p c n", p=P)
    wo_t = {}
    for c in range(4):
        wo_t[c] = wop.tile([P, 1024], F32R, tag="woa", bufs=4,
                           name=f"{name}_woA{c}")
        nc.sync.dma_start(out=wo_t[c], in_=wo_re[:, c, :])
    for q in range(4):
        for n in range(2):
            ps = wops.tile([P, 512], F32, tag="wop", bufs=2,
                           name=f"{name}_wopp{q}_{n}")
            for c in range(4):
                nc.tensor.matmul(ps, ot_sb[:, c, q * P:(q + 1) * P],
                                 wo_t[c][:, n * 512:(n + 1) * 512],
                                 start=(c == 0), stop=(c == 3))
            nc.vector.tensor_copy(out=partial_sb[:, q, n * 512:(n + 1) * 512],
                                  in_=ps)
            yield


def gen_wo_final(nc, t, name, ot_sb, wo_dram, partial_sb, res_fn, extra_bias,
                 g_name, b_name, dst_sb, eps_sb, wop, wops, post_q=None):
    """Wo chunks 4-7 + partial + residual (+bias) -> LN -> dst_sb (in place).
    post_q(q) emits per-q follow-up work (e.g. PE transposes) right after
    each LN so the PE has work while the next q's DVE chain runs."""
    wo_re = wo_dram.rearrange("(c p) n -> p c n", p=P)
    g_sb = wop.tile([P, 1024], F32, tag="g", name=f"{name}_g")
    nc.gpsimd.dma_start(out=g_sb, in_=_dram_bcast(t[g_name], P))
    b_sb = wop.tile([P, 1024], F32, tag="b", name=f"{name}_b")
    nc.gpsimd.dma_start(out=b_sb, in_=_dram_bcast(t[b_name], P))
    eb_sb = None
    if extra_bias is not None:
        eb_sb = wop.tile([P, 1024], F32, tag="eb", name=f"{name}_eb")
        nc.gpsimd.dma_start(out=eb_sb, in_=_dram_bcast(t[extra_bias], P))
    wo_t = {}
    for c in range(4, 8):
        wo_t[c] = wop.tile([P, 1024], F32R, tag="wob", bufs=4,
                           name=f"{name}_woB{c}")
        nc.sync.dma_start(out=wo_t[c], in_=wo_re[:, c, :])
    for q in range(4):
        for n in range(2):
            ps = wops.tile([P, 512], F32, tag="wof", bufs=2,
                           name=f"{name}_wofp{q}_{n}")
            for c in range(4, 8):
                nc.tensor.matmul(ps, ot_sb[:, c, q * P:(q + 1) * P],
                                 wo_t[c][:, n * 512:(n + 1) * 512],
                                 start=(c == 4), stop=(c == 7))
            dst = dst_sb[:, q, n * 512:(n + 1) * 512]
            nc.vector.tensor_tensor(
                out=dst, in0=ps, in1=partial_sb[:, q, n * 512:(n + 1) * 512],
                op=ALU.add)
            nc.vector.tensor_tensor(out=dst, in0=dst, in1=res_fn(q, n),
                                    op=ALU.add)
            if eb_sb is not None:
                nc.vector.tensor_tensor(
                    out=dst, in0=dst, in1=eb_sb[:, n * 512:(n + 1) * 512],
                    op=ALU.add)
            yield
        _layernorm_q(nc, wop, f"{name}_ln{q}", dst_sb, q, g_sb, b_sb,
                     dst_sb[:, q, :], eps_sb)
        yield
    if post_q is not None:
        for q in range(4):
            post_q(q)
            yield


def attention_block(nc, tc, t, *, name, srcT, wk, wv, qt_fill, qt_rate,
                    mask_sb, ones_sb, ones_f32, ot_sb, partial_sb, wo_dram):
    """Full attention for one block: qproj (via qt_fill callback), K/V
    projections, both attention halves (with interleaved fills), Wo partial.
    ot_sb/partial_sb live in the caller's pool."""
    with tc.tile_pool(name=f"{name}_qt", bufs=1) as qtp:
        qt_sb = qtp.tile([P, 8, QS], BF, tag="qt", name=f"{name}_qt")
        with tc.tile_pool(name=f"{name}_at0", bufs=1) as at0:
            kt0 = at0.tile([P, 4, S], BF, tag="kt", name=f"{name}_kt0")
            v0 = at0.tile([P, 16, 8, 65], BF, tag="v", name=f"{name}_v0")
            with tc.tile_pool(name=f"{name}_kv0", bufs=1) as kv0, \
                 tc.tile_pool(name=f"{name}_kvps0", bufs=1, space="PSUM") as kvps0:
                interleave(
                    gen_kvproj(nc, name, 0, kt0, v0, srcT, wk, wv,
                               kv0, kvps0, ones_f32),
                    qt_fill(qt_sb), rate=qt_rate)
            with tc.tile_pool(name=f"{name}_at1", bufs=1) as at1:
                kt1 = at1.tile([P, 4, S], BF, tag="kt", name=f"{name}_kt1")
                v1 = at1.tile([P, 16, 8, 65], BF, tag="v", name=f"{name}_v1")
                with tc.tile_pool(name=f"{name}_atps0", bufs=1,
                                  space="PSUM") as atps0:
                    with tc.tile_pool(name=f"{name}_kv1", bufs=1) as kv1, \
                         tc.tile_pool(name=f"{name}_kvps1", bufs=1,
                                      space="PSUM") as kvps1:
                        interleave(
                            gen_attn(nc, name, 0, kt0, v0, qt_sb, ot_sb,
                                     mask_sb, ones_sb, at1, atps0),
                            gen_kvproj(nc, name, 1, kt1, v1, srcT, wk, wv,
                                       kv1, kvps1, ones_f32),
                            rate=1.3)
                with tc.tile_pool(name=f"{name}_atps1", bufs=1,
                                  space="PSUM") as atps1:
                    with tc.tile_pool(name=f"{name}_wopar", bufs=1) as wop, \
                         tc.tile_pool(name=f"{name}_wops", bufs=1,
                                      space="PSUM") as wops:
                        interleave(
                            gen_attn(nc, name, 1, kt1, v1, qt_sb, ot_sb,
                                     mask_sb, ones_sb, at1, atps1),
                            gen_wo_partial(nc, name, ot_sb, wo_dram,
                                           partial_sb, wop, wops),
                            rate=0.12)


def build_nc(reps=1):
    nc = bacc.Bacc("TRN2", target_bir_lowering=False, debug=False,
                   enable_asserts=False, num_devices=NCORES)
    t = {}
    def din(name, shape, dt_=F32):
        t[name] = nc.dram_tensor(name, list(shape), dt_, kind="ExternalInput").ap()
    din("xqT", (D, QS), F32R); din("xq", (QS, D))
    din("xfT_bf", (D, S), BF); din("efT_bf", (D, S), BF)
    din("tmask", (P, 16)); din("smask", (P, 16))
    din("sa_wq", (D, INNER), F32R); din("ca_wq", (D, INNER), F32R)
    din("sa_wk_bf", (D, INNER), BF); din("sa_wv_bf", (D, INNER), BF)
    din("ca_wk_bf", (D, INNER), BF); din("ca_wv_bf", (D, INNER), BF)
    din("sa_wo", (INNER, D), F32R); din("ca_wo", (INNER, D), F32R)
    din("fc1_w", (D, FF), F32R); din("fc2_w", (FF, D), F32R)
    din("fc1_b", (P, 32)); din("fc2_b", (D,)); din("ca_bo", (D,))
    for v in ("ln1_g", "ln1_b", "ln2_g", "ln2_b", "ln3_g", "ln3_b"):
        din(v, (D,))
    out = nc.dram_tensor("out", [QS, D], F32, kind="ExternalOutput").ap()

    with tile.TileContext(nc) as tc:
      for _rep in range(reps):
        with tc.tile_pool(name="perm", bufs=1) as perm, \
             tc.tile_pool(name="keep", bufs=1) as keep:
            ident = perm.tile([P, P], F32, tag="ident", name="ident")
            make_identity(nc, ident)
            ones_f32 = perm.tile([P, P], F32, tag="ones32", name="ones_f32")
            nc.vector.memset(ones_f32, 1.0)
            ones_sb = perm.tile([1, 64], F32R, tag="ones", name="ones")
            nc.vector.tensor_copy(out=ones_sb, in_=ones_f32[0:1, 0:64])
            eps_sb = perm.tile([P, 1], F32, tag="eps", name="eps_sb")
            nc.vector.memset(eps_sb, EPS)
            tmask_sb = perm.tile([P, 16], F32, tag="tmask", name="tmask_sb")
            nc.sync.dma_start(out=tmask_sb, in_=t["tmask"])
            smask_sb = perm.tile([P, 16], F32, tag="smask", name="smask_sb")
            nc.sync.dma_start(out=smask_sb, in_=t["smask"])

            ln1_out = keep.tile([P, 4, 1024], F32, tag="ln1o", name="ln1_out")
            # ot/partial shared by SA and CA via slot rotation (bufs=1)
            sa_ot = keep.tile([P, 8, QS], F32R, tag="ot", name="sa_ot")
            sa_partial = keep.tile([P, 4, 1024], F32, tag="part", name="sa_part")

            # ================= self-attention =================
            def sa_qt_fill(qt_sb):
                with tc.tile_pool(name="sa_qin", bufs=1) as qin, \
                     tc.tile_pool(name="sa_qps", bufs=1, space="PSUM") as qps:
                    xqT_sb = qin.tile([P, 8, QS], F32R, tag="xqT",
                                      name="xqT_sb")
                    nc.sync.dma_start(
                        out=xqT_sb,
                        in_=t["xqT"].rearrange("(c p) s -> p c s", p=P))
                    yield from gen_qproj(nc, qt_sb, t["sa_wq"],
                                         lambda d: xqT_sb[:, d, :], "saq",
                                         qin, qps)

            attention_block(nc, tc, t, name="sa", srcT=t["xfT_bf"],
                            wk=t["sa_wk_bf"], wv=t["sa_wv_bf"],
                            qt_fill=sa_qt_fill, qt_rate=0.1,
                            mask_sb=tmask_sb, ones_sb=ones_sb,
                            ones_f32=ones_f32, ot_sb=sa_ot,
                            partial_sb=sa_partial, wo_dram=t["sa_wo"])

            # ================= cross-attention =================
            # CA ot/partial: fresh tiles on the same slots (WAR-rotated)
            ca_ot = keep.tile([P, 8, QS], F32R, tag="ot", name="ca_ot")
            ca_partial = keep.tile([P, 4, 1024], F32, tag="part",
                                   name="ca_part")

            def ca_qt_fill(qt_sb):
                # SA tail (Wo-final + LN1 + per-q t1 transposes) + CA qproj,
                # interleaved by the caller into CA's h0 K/V projection.
                with tc.tile_pool(name="sa_wof", bufs=1) as wop, \
                     tc.tile_pool(name="sa_wofps", bufs=1, space="PSUM") as wops, \
                     tc.tile_pool(name="ca_t1", bufs=1) as t1p, \
                     tc.tile_pool(name="ca_t1ps", bufs=1, space="PSUM") as tps:
                    xq_sb = wop.tile([P, 4, 1024], F32, tag="xq", name="xq_sb")
                    nc.sync.dma_start(
                        out=xq_sb,
                        in_=t["xq"].rearrange("(c p) d -> p c d", p=P))
                    ln1T = t1p.tile([P, 8, QS], F32R, tag="ln1T", name="ln1T")

                    def post_q(q):
                        for c in range(8):
                            pt = tps.tile([P, P], F32, tag="pt", bufs=2,
                                          name=f"t1_pt{c}_{q}")
                            nc.tensor.matmul(
                                pt, ln1_out[:, q, c * P:(c + 1) * P],
                                ident, is_transpose=True,
                                start=True, stop=True)
                            nc.vector.tensor_copy(
                                out=ln1T[:, c, q * P:(q + 1) * P], in_=pt)

                    yield from gen_wo_final(
                        nc, t, "sawo", sa_ot, t["sa_wo"], sa_partial,
                        lambda q, n: xq_sb[:, q, n * 512:(n + 1) * 512],
                        None, "ln1_g", "ln1_b", ln1_out, eps_sb, wop, wops,
                        post_q=post_q)
                    with tc.tile_pool(name="ca_qps", bufs=1,
                                      space="PSUM") as qps:
                        yield from gen_qproj(nc, qt_sb, t["ca_wq"],
                                             lambda d: ln1T[:, d, :], "caq",
                                             t1p, qps)

            attention_block(nc, tc, t, name="ca", srcT=t["efT_bf"],
                            wk=t["ca_wk_bf"], wv=t["ca_wv_bf"],
                            qt_fill=ca_qt_fill, qt_rate=0.35,
                            mask_sb=smask_sb, ones_sb=ones_sb,
                            ones_f32=ones_f32, ot_sb=ca_ot,
                            partial_sb=ca_partial, wo_dram=t["ca_wo"])

            # ===== CA tail: Wo-final -> LN2 -> per-q t2 transposes; then MLP
            with tc.tile_pool(name="mid", bufs=1) as mid:
                ln2_out = mid.tile([P, 4, 1024], F32, tag="ln2o",
                                   name="ln2_out")
                ln2T = mid.tile([P, 8, QS], F32R, tag="ln2T", name="ln2T")
                with tc.tile_pool(name="ca_wof", bufs=1) as wop, \
                     tc.tile_pool(name="ca_wofps", bufs=1,
                                  space="PSUM") as wops, \
                     tc.tile_pool(name="t2ps", bufs=1, space="PSUM") as tps2:

                    def post_q2(q):
                        for c in range(8):
                            pt = tps2.tile([P, P], F32, tag="pt", bufs=2,
                                           name=f"t2_pt{c}_{q}")
                            nc.tensor.matmul(
                                pt, ln2_out[:, q, c * P:(c + 1) * P],
                                ident, is_transpose=True,
                                start=True, stop=True)
                            nc.vector.tensor_copy(
                                out=ln2T[:, c, q * P:(q + 1) * P], in_=pt)

                    drain(gen_wo_final(
                        nc, t, "cawo", ca_ot, t["ca_wo"], ca_partial,
                        lambda q, n: ln1_out[:, q, n * 512:(n + 1) * 512],
                        "ca_bo", "ln2_g", "ln2_b", ln2_out, eps_sb,
                        wop, wops, post_q=post_q2))

                # ================= MLP =================
                with tc.tile_pool(name="mlp", bufs=1) as mlp:
                    h2T = mlp.tile([P, 32, QS], F32R, tag="h2T", name="h2T")
                    fc1b_sb = mlp.tile([P, 32], F32, tag="f1b",
                                       name="fc1b_sb")
                    nc.sync.dma_start(out=fc1b_sb, in_=t["fc1_b"])
                    fc1_re = t["fc1_w"].rearrange("(c p) n -> p c n", p=P)
                    with tc.tile_pool(name="fc1", bufs=1) as f1p, \
                         tc.tile_pool(name="fc1ps", bufs=1,
                                      space="PSUM") as f1ps:
                        for m in range(32):
                            w_m = f1p.tile([P, 8, P], F32R, tag="w",
                                           bufs=3, name=f"fc1w_{m}")
                            nc.sync.dma_start(
                                out=w_m, in_=fc1_re[:, :, m * P:(m + 1) * P])
                            psf = f1ps.tile([P, QS], F32, tag="ps", bufs=2,
                                            name=f"fc1ps_{m}")
                            for d in range(8):
                                nc.tensor.matmul(psf, w_m[:, d, :],
                                                 ln2T[:, d, :],
                                                 start=(d == 0),
                                                 stop=(d == 7))
                            nc.scalar.activation(
                                out=h2T[:, m, :], in_=psf, func=AF.Gelu,
                                bias=fc1b_sb[:, m:m + 1], scale=1.0)
                    fc2_re = t["fc2_w"].rearrange("(c p) n -> p c n", p=P)
                    with tc.tile_pool(name="fc2", bufs=1) as f2p, \
                         tc.tile_pool(name="fc2ps", bufs=1,
                                      space="PSUM") as f2ps:
                        fc2b_sb = f2p.tile([P, 1024], F32, tag="f2b",
                                           name="fc2b_sb")
                        nc.gpsimd.dma_start(out=fc2b_sb,
                                            in_=_dram_bcast(t["fc2_b"], P))
                        ln3g_sb = f2p.tile([P, 1024], F32, tag="l3g",
                                           name="ln3g_sb")
                        nc.gpsimd.dma_start(out=ln3g_sb,
                                            in_=_dram_bcast(t["ln3_g"], P))
                        ln3b_sb = f2p.tile([P, 1024], F32, tag="l3b",
                                           name="ln3b_sb")
                        nc.gpsimd.dma_start(out=ln3b_sb,
                                            in_=_dram_bcast(t["ln3_b"], P))
                        ps2 = [[f2ps.tile([P, 512], F32, tag=f"pr{q}_{n}",
                                          name=f"fc2ps{q}_{n}")
                                for n in range(2)] for q in range(4)]
                        out_re = out.rearrange("(c p) d -> p c d", p=P)
                        for fb in range(8):
                            wb = f2p.tile([P, 4, 1024], F32R, tag="wb",
                                          bufs=2, name=f"fc2w_{fb}")
                            nc.sync.dma_start(
                                out=wb,
                                in_=fc2_re[:, fb * 4:(fb + 1) * 4, :])
                            for q in range(4):
                                for n in range(2):
                                    for f in range(4):
                                        ff = fb * 4 + f
                                        nc.tensor.matmul(
                                            ps2[q][n],
                                            h2T[:, ff, q * P:(q + 1) * P],
                                            wb[:, f, n * 512:(n + 1) * 512],
                                            start=(ff == 0),
                                            stop=(ff == 31))
                                if fb == 7:
                                    # q's accumulation done: residual + LN3
                                    # overlap remaining q's matmuls
                                    for n in range(2):
                                        nc.vector.tensor_tensor(
                                            out=ln2_out[:, q, n * 512:(n + 1) * 512],
                                            in0=ps2[q][n],
                                            in1=ln2_out[:, q, n * 512:(n + 1) * 512],
                                            op=ALU.add)
                                    nc.vector.tensor_tensor(
                                        out=ln2_out[:, q, :],
                                        in0=ln2_out[:, q, :],
                                        in1=fc2b_sb, op=ALU.add)
                                    o_sb = f2p.tile([P, 1024], F32, tag="osb",
                                                    bufs=2, name=f"out_sb{q}")
                                    _layernorm_q(nc, f2p, f"ln3_{q}", ln2_out,
                                                 q, ln3g_sb, ln3b_sb, o_sb,
                                                 eps_sb)
                                    nc.sync.dma_start(out=out_re[:, q, :],
                                                      in_=o_sb)
    nc.compile()
    return nc


_NC_CACHE = None
_RUNNER_CACHE = None


def _get_nc():
    global _NC_CACHE
    if _NC_CACHE is None:
        _NC_CACHE = build_nc()
    return _NC_CACHE


def _build_runner(nc, n_cores):
    """Cached jitted SPMD executor (same machinery run_bass_kernel_spmd uses
    under axon, but reusable across kernel() calls)."""
    import jax
    from jax.sharding import Mesh, PartitionSpec
    from jax.experimental.shard_map import shard_map
    from concourse import bass2jax

    bass2jax.install_neuronx_cc_hook()
    part_name = nc.partition_id_tensor.name if nc.partition_id_tensor else None
    in_names, out_names, out_avals, zero_shapes = [], [], [], []
    for alloc in nc.m.functions[0].allocations:
        if not isinstance(alloc, mybir.MemoryLocationSet):
            continue
        name = alloc.memorylocations[0].name
        if alloc.kind == "ExternalInput":
            if name != part_name:
                in_names.append(name)
        elif alloc.kind == "ExternalOutput":
            out_names.append(name)
            shape = tuple(alloc.tensor_shape)
            dtype = mybir.dt.np(alloc.dtype)
            out_avals.append(jax.core.ShapedArray(shape, dtype))
            zero_shapes.append((shape, dtype))
    n_params = len(in_names)
    all_names = list(in_names) + list(out_names)
    if part_name is not None:
        all_names.append(part_name)

    def _body(*args):
        operands = list(args)
        if part_name is not None:
            operands.append(bass2jax.partition_id_tensor())
        outs = bass2jax._bass_exec_p.bind(
            *operands, out_avals=tuple(out_avals), in_names=tuple(all_names),
            out_names=tuple(out_names), lowering_input_output_aliases=(),
            sim_require_finite=True, sim_require_nnan=True, nc=nc)
        return tuple(outs)

    devices = jax.devices()[:n_cores]
    mesh = Mesh(np.asarray(devices), ("core",))
    in_specs = (PartitionSpec("core"),) * (n_params + len(out_names))
    out_specs = (PartitionSpec("core"),) * len(out_names)
    fn = jax.jit(shard_map(_body, mesh=mesh, in_specs=in_specs,
                           out_specs=out_specs, check_rep=False),
                 keep_unused=True)
    sharding = jax.sharding.NamedSharding(mesh, PartitionSpec("core"))
    return fn, in_names, out_names, out_avals, zero_shapes, sharding


def _run_fast(in_maps):
    """Execute via the cached jitted runner. Returns per-core out dicts."""
    global _RUNNER_CACHE
    import jax
    nc = _get_nc()
    if _RUNNER_CACHE is None:
        _RUNNER_CACHE = _build_runner(nc, NCORES)
    fn, in_names, out_names, out_avals, zero_shapes, sharding = _RUNNER_CACHE
    concat_in = [
        np.concatenate([np.asarray(in_maps[c][nm]) for c in range(NCORES)],
                       axis=0)
        for nm in in_names
    ]
    concat_zeros = [np.zeros((NCORES * s[0], *s[1:]), d)
                    for (s, d) in zero_shapes]
    dev_in = [jax.device_put(a, sharding) for a in concat_in]
    dev_z = [jax.device_put(a, sharding) for a in concat_zeros]
    outs = fn(*dev_in, *dev_z)
    jax.block_until_ready(outs)
    return [
        {nm: np.asarray(outs[i]).reshape(NCORES, *out_avals[i].shape)[c]
         for i, nm in enumerate(out_names)}
        for c in range(NCORES)
    ]


def make_in_maps(inputs):
    f32c = lambda a: np.ascontiguousarray(np.asarray(a), dtype=np.float32)
    bfc = lambda a: np.ascontiguousarray(np.asarray(a, dtype=np.float32)
                                         .astype(ml_dtypes.bfloat16))
    x = f32c(inputs["x"]); enc = f32c(inputs["enc_output"])
    tgt = np.asarray(inputs["tgt_mask"]); src = np.asarray(inputs["src_mask"])

    shared = {
        "sa_wq": f32c(inputs["sa_wq"]) * np.float32(SCALE),
        "ca_wq": f32c(inputs["ca_wq"]) * np.float32(SCALE),
        "sa_wk_bf": bfc(inputs["sa_wk"]), "sa_wv_bf": bfc(inputs["sa_wv"]),
        "ca_wk_bf": bfc(inputs["ca_wk"]), "ca_wv_bf": bfc(inputs["ca_wv"]),
        "sa_wo": f32c(inputs["sa_wo"]), "ca_wo": f32c(inputs["ca_wo"]),
        "fc1_w": f32c(inputs["fc1_w"]), "fc2_w": f32c(inputs["fc2_w"]),
        "fc1_b": np.ascontiguousarray(f32c(inputs["fc1_b"]).reshape(32, P).T),
        "fc2_b": f32c(inputs["fc2_b"]), "ca_bo": f32c(inputs["ca_bo"]),
        "ln1_g": f32c(inputs["ln1_g"]), "ln1_b": f32c(inputs["ln1_b"]),
        "ln2_g": f32c(inputs["ln2_g"]), "ln2_b": f32c(inputs["ln2_b"]),
        "ln3_g": f32c(inputs["ln3_g"]), "ln3_b": f32c(inputs["ln3_b"]),
    }
    shared = {k: np.ascontiguousarray(v) for k, v in shared.items()}

    in_maps = []
    for c in range(NCORES):
        b, qi = c // 4, c % 4
        q0 = qi * QS
        xb = x[b]; eb = enc[b]
        tm = np.where(tgt[b], np.float32(NEG), np.float32(0.0)).astype(np.float32)
        sm = np.where(src[b], np.float32(NEG), np.float32(0.0)).astype(np.float32)
        m = dict(shared)
        m["xqT"] = np.ascontiguousarray(xb[q0:q0 + QS].T)
        m["xq"] = np.ascontiguousarray(xb[q0:q0 + QS] + f32c(inputs["sa_bo"]))
        m["xfT_bf"] = np.ascontiguousarray(xb.T.astype(ml_dtypes.bfloat16))
        m["efT_bf"] = np.ascontiguousarray(eb.T.astype(ml_dtypes.bfloat16))
        m["tmask"] = np.ascontiguousarray(tm.reshape(16, P).T)
        m["smask"] = np.ascontiguousarray(sm.reshape(16, P).T)
        in_maps.append(m)
    return in_maps


def kernel(**inputs):
    in_maps = make_in_maps(inputs)
    try:
        results = _run_fast(in_maps)
    except Exception:
        nc = _get_nc()
        results = run_bass_kernel_spmd(nc, in_maps,
                                       core_ids=list(range(NCORES))).results
    out = np.empty((B, S, D), np.float32)
    for c in range(NCORES):
        b, qi = c // 4, c % 4
        out[b, qi * QS:(qi + 1) * QS] = results[c]["out"]
    return out



# revision 9
# speedup vs baseline: 49.8518x; 49.8518x over previous
"""v3: fp8-DoubleRow transformer block kernel for TRN2 (8 cores).

Sharding: core c handles batch c//4, query chunk (c%4)*512, full K/V
duplicated per batch (no collectives).

vs v2:
- All attention matmuls except QK^T scores run fp8(e4m3) DoubleRow
  (2 contraction chunks per instruction, 2x PE rate): Q/K/V projections,
  AV (attention @ V), Wo. Scores stay bf16 (DH=64 cannot pair).
- Scale plumbing keeps everything in fp8 range with zero extra ops:
  weights x16 in fp8, exp(s-2) shift baked into the mask bias, V ones-row
  = 16, ot stored x64 (ones_sb = 64), Wo evacuated with x1/1024.
- MLP stays bf16 (fp8 there breaks the 2e-2 tolerance; bf16 matches f32r
  PE speed and halves weight DMA).
"""

import os
import sys

import numpy as np
import ml_dtypes

for _p in ("/opt/trn_rl_repo", "/root/.axon_site/_ro/trn_rl_repo"):
    if os.path.isdir(_p) and _p not in sys.path:
        sys.path.append(_p)

import concourse.bass as bass
import concourse.mybir as mybir
import concourse.tile as tile
from concourse import bacc
from concourse.bass_utils import run_bass_kernel_spmd
from concourse.masks import make_identity

B, S, D = 2, 2048, 1024
H, DH = 16, 64
INNER = H * DH
FF = 4096
SCALE = DH ** -0.5
EPS = 1e-5
QS = 512
P = 128
NEG = -1e9
NCORES = 8
WSC = 16.0            # fp8 weight pre-scale
OSC = 64.0            # ot (attention output) fp8 scale
SHIFT = -2.0          # exp(s + SHIFT) to stay under fp8 e4m3 max (240)
SCALE_EFF = SCALE / (WSC * WSC)   # exp scale: scores carry qx16 * kx16

F32 = mybir.dt.float32
F32R = mybir.dt.float32r
BF = mybir.dt.bfloat16
F8 = mybir.dt.float8e4
AF = mybir.ActivationFunctionType
ALU = mybir.AluOpType
DR = mybir.MatmulPerfMode.DoubleRow

_SENT = object()


def drain(gen):
    for _ in gen:
        pass


def interleave(primary, filler, rate):
    acc = 0.0
    for _ in primary:
        if filler is None:
            continue
        acc += rate
        while acc >= 1.0:
            acc -= 1.0
            if next(filler, _SENT) is _SENT:
                filler = None
                break
    if filler is not None:
        drain(filler)


def _dram_bcast(vec_ap, parts):
    return bass.AP(tensor=vec_ap.tensor, offset=vec_ap.offset,
                   ap=[[0, parts]] + list(vec_ap.ap))


def _layernorm_q(nc, pool, name, h_sb, q, g_sb, b_sb, dst, eps_sb):
    stats = pool.tile([P, 2, 6], F32, tag="lnst", bufs=2, name=f"{name}_st")
    for sub in range(2):
        nc.vector.bn_stats(out=stats[:, sub, :],
                           in_=h_sb[:, q, sub * 512:(sub + 1) * 512])
    mv = pool.tile([P, 2], F32, tag="lnmv", bufs=2, name=f"{name}_mv")
    nc.vector.bn_aggr(out=mv, in_=stats)
    std = pool.tile([P, 1], F32, tag="lnstd", bufs=2, name=f"{name}_sd")
    nc.scalar.activation(out=std, in_=mv[:, 1:2], func=AF.Sqrt, bias=eps_sb)
    rstd = pool.tile([P, 1], F32, tag="lnrs", bufs=2, name=f"{name}_rs")
    nc.vector.reciprocal(rstd, std)
    nc.vector.tensor_scalar(out=dst, in0=h_sb[:, q, :], scalar1=mv[:, 0:1],
                            scalar2=rstd, op0=ALU.subtract, op1=ALU.mult)
    nc.gpsimd.tensor_tensor(out=dst, in0=dst, in1=g_sb, op=ALU.mult)
    nc.gpsimd.tensor_tensor(out=dst, in0=dst, in1=b_sb, op=ALU.add)


def gen_qproj(nc, qt_sb, w_dram, rhs_fn, wtag, pool, psp):
    """qt_sb[:, m, :] (bf16, 16x-scaled) = w8[d, m].T @ rhs8(d); DoubleRow."""
    w_re = w_dram.rearrange("(c p) n -> p c n", p=P)
    for m in range(8):
        w_m = pool.tile([P, 8, P], F8, tag="qp_w", bufs=3, name=f"qpw_{wtag}{m}")
        nc.sync.dma_start(out=w_m, in_=w_re[:, :, m * P:(m + 1) * P])
        ps = psp.tile([P, QS], F32, tag="qp_ps", bufs=2, name=f"qpps_{wtag}{m}")
        for t in range(4):
            nc.tensor.matmul(ps, w_m[:, 2 * t:2 * t + 2, :], rhs_fn(t),
                             start=(t == 0), stop=(t == 3), perf_mode=DR)
        nc.vector.tensor_copy(out=qt_sb[:, m, :], in_=ps)
        yield


def gen_kvproj(nc, name, hf, kt_sb, v_sb, srcT_dram, wk_dram, wv_dram,
               kvp, kvps, c16_f32):
    """K^T/V projection for half hf, fp8 DoubleRow.
    kt_sb [128, 4, 2048] bf16 (16x); v_sb [128, 16, 8, 72] fp8 (16x)."""
    NSL = 512
    wk_re = wk_dram.rearrange("(c p) n -> p c n", p=P)
    wv_re = wv_dram.rearrange("(c p) n -> p c n", p=P)
    srcT_re = srcT_dram.rearrange("(c p) s -> p c s", p=P)
    hi = hf * 512
    wk_h = kvp.tile([P, 8, 512], F8, tag="wk", name=f"{name}_wk{hf}")
    wv_h = kvp.tile([P, 8, 512], F8, tag="wv", name=f"{name}_wv{hf}")
    for d in range(8):
        nc.sync.dma_start(out=wk_h[:, d, :], in_=wk_re[:, d, hi:hi + 512])
        nc.sync.dma_start(out=wv_h[:, d, :], in_=wv_re[:, d, hi:hi + 512])
    for s in range(S // NSL):
        xf_s = kvp.tile([P, 8, NSL], F8, tag="xf", bufs=2,
                        name=f"{name}_xf{hf}_{s}")
        for d in range(8):
            nc.sync.dma_start(out=xf_s[:, d, :],
                              in_=srcT_re[:, d, s * NSL:(s + 1) * NSL])
        for m in range(4):
            ps = kvps.tile([P, NSL], F32, tag="kv_ps", bufs=2,
                           name=f"{name}_psk{hf}_{s}_{m}")
            for t in range(4):
                nc.tensor.matmul(ps, wk_h[:, 2 * t:2 * t + 2,
                                          m * P:(m + 1) * P],
                                 xf_s[:, 2 * t:2 * t + 2, :],
                                 start=(t == 0), stop=(t == 3), perf_mode=DR)
                if t % 2 == 1:
                    yield
            nc.vector.tensor_copy(out=kt_sb[:, m, s * NSL:(s + 1) * NSL],
                                  in_=ps)
        for c in range(NSL // P):
            ps = kvps.tile([P, 512], F32, tag="kv_ps", bufs=2,
                           name=f"{name}_psv{hf}_{s}_{c}")
            for t in range(4):
                nc.tensor.matmul(ps, xf_s[:, 2 * t:2 * t + 2,
                                          c * P:(c + 1) * P],
                                 wv_h[:, 2 * t:2 * t + 2, :],
                                 start=(t == 0), stop=(t == 3), perf_mode=DR)
                if t % 2 == 1:
                    yield
            nc.vector.tensor_copy(
                out=v_sb[:, s * (NSL // P) + c, :, 0:64],
                in_=ps.rearrange("p (h e) -> p h e", h=8))
    nc.vector.tensor_copy(
        out=v_sb[:, :, :, 64:65],
        in_=c16_f32[:, 0:128].rearrange("p (a b c) -> p a b c", a=16, b=8))
    yield


def gen_attn(nc, name, hf, kt_sb, v_sb, qt_sb, ot_sb, mask_sb, ones_sb,
             misc, atps):
    """One quantum per (h, j) step; AV in fp8-DoubleRow j-chunk pairs,
    lagging one pair; divide chains inline."""
    psum_o = {}
    prev = None
    pending_div = None

    def divide_chain(h_loc):
        h_glob = hf * 8 + h_loc
        g = h_glob // 2
        poff = (h_glob % 2) * 64
        po = psum_o.pop(h_loc)
        rcp = misc.tile([1, QS], F32R, tag="rcp", bufs=2,
                        name=f"{name}_rcp{hf}_{h_loc}")
        with nc.allow_low_precision(reason="f32r softmax denom"):
            nc.vector.reciprocal(rcp, po[64:65, :])
        psb = atps.tile([64, QS], F32, tag="psb", bufs=1,
                        name=f"{name}_psb{hf}_{h_loc}")
        nc.tensor.matmul(psb, ones_sb[0:1, :], rcp, start=True, stop=True)
        rb = misc.tile([64, QS], F32, tag="rb", bufs=2,
                       name=f"{name}_rb{hf}_{h_loc}")
        nc.vector.tensor_copy(out=rb, in_=psb)
        if poff == 0:
            nc.vector.tensor_tensor(out=ot_sb[0:64, g, :], in0=po[0:64, :],
                                    in1=rb, op=ALU.mult)
        else:
            tmp = misc.tile([64, QS], F8, tag="tdiv", bufs=2,
                            name=f"{name}_td{hf}_{h_loc}")
            nc.vector.tensor_tensor(out=tmp, in0=po[0:64, :], in1=rb,
                                    op=ALU.mult)
            nc.sync.dma_start(out=ot_sb[64:128, g, :], in_=tmp)

    for h_loc in range(8):
        h_glob = hf * 8 + h_loc
        m_loc = h_loc // 2
        poff = (h_loc % 2) * 64
        g = h_glob // 2
        psum_o[h_loc] = atps.tile([65, QS], F32, tag="pso", bufs=2,
                                  name=f"{name}_pso{hf}_{h_loc}")
        for jj in range(8):
            et2 = misc.tile([P, 2, QS], F8, tag="et", bufs=3,
                            name=f"{name}_et{hf}_{h_loc}_{jj}")
            for sub in range(2):
                j = 2 * jj + sub
                ps_s = atps.tile([P, QS], F32, tag="pss", bufs=3,
                                 name=f"{name}_pss{hf}_{h_loc}_{j}")
                nc.tensor.matmul(ps_s,
                                 kt_sb[poff:poff + 64, m_loc,
                                       j * P:(j + 1) * P],
                                 qt_sb[poff:poff + 64, g, :],
                                 start=True, stop=True,
                                 tile_position=(poff, 0))
                nc.scalar.activation(out=et2[:, sub, :], in_=ps_s,
                                     func=AF.Exp,
                                     bias=mask_sb[:, j:j + 1],
                                     scale=SCALE_EFF)
                if sub == 1:
                    if prev is not None:
                        ph, pjj, pet = prev
                        nc.tensor.matmul(psum_o[ph],
                                         v_sb[:, 2 * pjj:2 * pjj + 2, ph,
                                              0:65],
                                         pet, start=(pjj == 0),
                                         stop=(pjj == 7), perf_mode=DR)
                    prev = (h_loc, jj, et2)
                if pending_div is not None and jj == 2 and sub == 0:
                    divide_chain(pending_div)
                    pending_div = None
                yield
        if h_loc == 7:
            ph, pjj, pet = prev
            nc.tensor.matmul(psum_o[ph],
                             v_sb[:, 2 * pjj:2 * pjj + 2, ph, 0:65],
                             pet, start=(pjj == 0), stop=(pjj == 7),
                             perf_mode=DR)
            prev = None
            if pending_div is not None:
                divide_chain(pending_div)
            divide_chain(7)
        else:
            pending_div = h_loc


def gen_wo_partial(nc, name, ot_sb, wo_dram, partial_sb, wop, wops):
    """Wo contraction over inner chunks 0-3 (fp8 DR) -> partial_sb f32."""
    wo_re = wo_dram.rearrange("(c p) n -> p c n", p=P)
    wo_t = wop.tile([P, 4, 1024], F8, tag="woa", name=f"{name}_woA")
    nc.sync.dma_start(out=wo_t, in_=wo_re[:, 0:4, :])
    for q in range(4):
        for n in range(2):
            ps = wops.tile([P, 512], F32, tag="wop", bufs=2,
                           name=f"{name}_wopp{q}_{n}")
            for t in range(2):
                nc.tensor.matmul(ps,
                                 ot_sb[:, 2 * t:2 * t + 2, q * P:(q + 1) * P],
                                 wo_t[:, 2 * t:2 * t + 2,
                                      n * 512:(n + 1) * 512],
                                 start=(t == 0), stop=(t == 1), perf_mode=DR)
            nc.vector.tensor_scalar_mul(
                partial_sb[:, q, n * 512:(n + 1) * 512], ps,
                1.0 / (WSC * OSC))
            yield


def gen_wo_final(nc, t, name, ot_sb, wo_dram, partial_sb, res_fn, extra_bias,
                 g_name, b_name, dst_sb, eps_sb, wop, wops, post_q=None):
    """Wo chunks 4-7 (fp8 DR) + partial + residual (+bias) -> LN -> dst_sb."""
    wo_re = wo_dram.rearrange("(c p) n -> p c n", p=P)
    g_sb = wop.tile([P, 1024], F32, tag="g", name=f"{name}_g")
    nc.gpsimd.dma_start(out=g_sb, in_=_dram_bcast(t[g_name], P))
    b_sb = wop.tile([P, 1024], F32, tag="b", name=f"{name}_b")
    nc.gpsimd.dma_start(out=b_sb, in_=_dram_bcast(t[b_name], P))
    eb_sb = None
    if extra_bias is not None:
        eb_sb = wop.tile([P, 1024], F32, tag="eb", name=f"{name}_eb")
        nc.gpsimd.dma_start(out=eb_sb, in_=_dram_bcast(t[extra_bias], P))
    wo_t = wop.tile([P, 4, 1024], F8, tag="wob", name=f"{name}_woB")
    nc.sync.dma_start(out=wo_t, in_=wo_re[:, 4:8, :])
    for q in range(4):
        for n in range(2):
            ps = wops.tile([P, 512], F32, tag="wof", bufs=2,
                           name=f"{name}_wofp{q}_{n}")
            for t2 in range(2):
                nc.tensor.matmul(ps,
                                 ot_sb[:, 4 + 2 * t2:6 + 2 * t2,
                                       q * P:(q + 1) * P],
                                 wo_t[:, 2 * t2:2 * t2 + 2,
                                      n * 512:(n + 1) * 512],
                                 start=(t2 == 0), stop=(t2 == 1),
                                 perf_mode=DR)
            dst = dst_sb[:, q, n * 512:(n + 1) * 512]
            nc.vector.tensor_scalar_mul(dst, ps, 1.0 / (WSC * OSC))
            nc.vector.tensor_tensor(
                out=dst, in0=dst, in1=partial_sb[:, q, n * 512:(n + 1) * 512],
                op=ALU.add)
            nc.vector.tensor_tensor(out=dst, in0=dst, in1=res_fn(q, n),
                                    op=ALU.add)
            if eb_sb is not None:
                nc.vector.tensor_tensor(
                    out=dst, in0=dst, in1=eb_sb[:, n * 512:(n + 1) * 512],
                    op=ALU.add)
            yield
        _layernorm_q(nc, wop, f"{name}_ln{q}", dst_sb, q, g_sb, b_sb,
                     dst_sb[:, q, :], eps_sb)
        yield
    if post_q is not None:
        for q in range(4):
            post_q(q)
            yield


def attention_block(nc, tc, t, *, name, srcT, wk, wv, qt_fill, qt_rate,
                    mask_sb, ones_sb, c16_f32, ot_sb, partial_sb, wo_dram):
    """Full attention for one block: qproj (via qt_fill callback), K/V
    projections, both attention halves (with interleaved fills), Wo partial."""
    with tc.tile_pool(name=f"{name}_qt", bufs=1) as qtp:
        qt_sb = qtp.tile([P, 8, QS], BF, tag="qt", name=f"{name}_qt")
        with tc.tile_pool(name=f"{name}_at0", bufs=1) as at0:
            kt0 = at0.tile([P, 4, S], BF, tag="kt", name=f"{name}_kt0")
            v0 = at0.tile([P, 16, 8, 72], F8, tag="v", name=f"{name}_v0")
            with tc.tile_pool(name=f"{name}_kv0", bufs=1) as kv0, \
                 tc.tile_pool(name=f"{name}_kvps0", bufs=1, space="PSUM") as kvps0:
                interleave(
                    gen_kvproj(nc, name, 0, kt0, v0, srcT, wk, wv,
                               kv0, kvps0, c16_f32),
                    qt_fill(qt_sb), rate=qt_rate)
            with tc.tile_pool(name=f"{name}_at1", bufs=1) as at1:
                kt1 = at1.tile([P, 4, S], BF, tag="kt", name=f"{name}_kt1")
                v1 = at1.tile([P, 16, 8, 72], F8, tag="v", name=f"{name}_v1")
                with tc.tile_pool(name=f"{name}_atps0", bufs=1,
                                  space="PSUM") as atps0:
                    with tc.tile_pool(name=f"{name}_kv1", bufs=1) as kv1, \
                         tc.tile_pool(name=f"{name}_kvps1", bufs=1,
                                      space="PSUM") as kvps1:
                        interleave(
                            gen_attn(nc, name, 0, kt0, v0, qt_sb, ot_sb,
                                     mask_sb, ones_sb, at1, atps0),
                            gen_kvproj(nc, name, 1, kt1, v1, srcT, wk, wv,
                                       kv1, kvps1, c16_f32),
                            rate=1.3)
                with tc.tile_pool(name=f"{name}_atps1", bufs=1,
                                  space="PSUM") as atps1:
                    with tc.tile_pool(name=f"{name}_wopar", bufs=1) as wop, \
                         tc.tile_pool(name=f"{name}_wops", bufs=1,
                                      space="PSUM") as wops:
                        interleave(
                            gen_attn(nc, name, 1, kt1, v1, qt_sb, ot_sb,
                                     mask_sb, ones_sb, at1, atps1),
                            gen_wo_partial(nc, name, ot_sb, wo_dram,
                                           partial_sb, wop, wops),
                            rate=0.12)


def build_nc(reps=1):
    nc = bacc.Bacc("TRN2", target_bir_lowering=False, debug=False,
                   enable_asserts=False, num_devices=NCORES)
    t = {}
    def din(name, shape, dt_=F32):
        t[name] = nc.dram_tensor(name, list(shape), dt_, kind="ExternalInput").ap()
    din("xqT8", (D, QS), F8); din("xq", (QS, D))
    din("xfT8", (D, S), F8); din("efT8", (D, S), F8)
    din("tmask", (P, 16)); din("smask", (P, 16))
    din("sa_wq8", (D, INNER), F8); din("ca_wq8", (D, INNER), F8)
    din("sa_wk8", (D, INNER), F8); din("sa_wv8", (D, INNER), F8)
    din("ca_wk8", (D, INNER), F8); din("ca_wv8", (D, INNER), F8)
    din("sa_wo8", (INNER, D), F8); din("ca_wo8", (INNER, D), F8)
    din("fc1_w", (D, FF), BF); din("fc2_w", (FF, D), BF)
    din("fc1_b", (P, 32)); din("fc2_b", (D,)); din("ca_bo", (D,))
    for v in ("ln1_g", "ln1_b", "ln2_g", "ln2_b", "ln3_g", "ln3_b"):
        din(v, (D,))
    out = nc.dram_tensor("out", [QS, D], F32, kind="ExternalOutput").ap()

    with tile.TileContext(nc) as tc:
      for _rep in range(reps):
        with tc.tile_pool(name="perm", bufs=1) as perm, \
             tc.tile_pool(name="keep", bufs=1) as keep:
            ident = perm.tile([P, P], F32, tag="ident", name="ident")
            make_identity(nc, ident)
            c16_f32 = perm.tile([P, P], F32, tag="c16", name="c16_f32")
            nc.vector.memset(c16_f32, WSC)
            ones_sb = perm.tile([1, 64], F32R, tag="ones", name="ones")
            nc.vector.tensor_scalar_mul(ones_sb, c16_f32[0:1, 0:64],
                                        OSC / WSC)
            eps_sb = perm.tile([P, 1], F32, tag="eps", name="eps_sb")
            nc.vector.memset(eps_sb, EPS)
            tmask_sb = perm.tile([P, 16], F32, tag="tmask", name="tmask_sb")
            nc.sync.dma_start(out=tmask_sb, in_=t["tmask"])
            smask_sb = perm.tile([P, 16], F32, tag="smask", name="smask_sb")
            nc.sync.dma_start(out=smask_sb, in_=t["smask"])

            ln1_out = keep.tile([P, 4, 1024], F32, tag="ln1o", name="ln1_out")
            # ot/partial shared by SA and CA via slot rotation (bufs=1)
            sa_ot = keep.tile([P, 8, QS], F8, tag="ot", name="sa_ot")
            sa_partial = keep.tile([P, 4, 1024], F32, tag="part", name="sa_part")

            # ================= self-attention =================
            def sa_qt_fill(qt_sb):
                with tc.tile_pool(name="sa_qin", bufs=1) as qin, \
                     tc.tile_pool(name="sa_qps", bufs=1, space="PSUM") as qps:
                    xqT_sb = qin.tile([P, 8, QS], F8, tag="xqT",
                                      name="xqT_sb")
                    nc.sync.dma_start(
                        out=xqT_sb,
                        in_=t["xqT8"].rearrange("(c p) s -> p c s", p=P))
                    yield from gen_qproj(
                        nc, qt_sb, t["sa_wq8"],
                        lambda tt: xqT_sb[:, 2 * tt:2 * tt + 2, :], "saq",
                        qin, qps)

            attention_block(nc, tc, t, name="sa", srcT=t["xfT8"],
                            wk=t["sa_wk8"], wv=t["sa_wv8"],
                            qt_fill=sa_qt_fill, qt_rate=0.1,
                            mask_sb=tmask_sb, ones_sb=ones_sb,
                            c16_f32=c16_f32,
                            ot_sb=sa_ot, partial_sb=sa_partial,
                            wo_dram=t["sa_wo8"])

            # ================= cross-attention =================
            ca_ot = keep.tile([P, 8, QS], F8, tag="ot", name="ca_ot")
            ca_partial = keep.tile([P, 4, 1024], F32, tag="part",
                                   name="ca_part")

            def ca_qt_fill(qt_sb):
                # SA tail (Wo-final + LN1 + per-q t1 transposes) + CA qproj,
                # interleaved by the caller into CA's h0 K/V projection.
                with tc.tile_pool(name="sa_wof", bufs=1) as wop, \
                     tc.tile_pool(name="sa_wofps", bufs=1, space="PSUM") as wops, \
                     tc.tile_pool(name="ca_t1", bufs=1) as t1p, \
                     tc.tile_pool(name="ca_t1ps", bufs=1, space="PSUM") as tps:
                    xq_sb = wop.tile([P, 4, 1024], F32, tag="xq", name="xq_sb")
                    nc.sync.dma_start(
                        out=xq_sb,
                        in_=t["xq"].rearrange("(c p) d -> p c d", p=P))
                    ln1T = t1p.tile([P, 8, QS], F8, tag="ln1T", name="ln1T")

                    def post_q(q):
                        for c in range(8):
                            pt = tps.tile([P, P], F32, tag="pt", bufs=2,
                                          name=f"t1_pt{c}_{q}")
                            nc.tensor.matmul(
                                pt, ln1_out[:, q, c * P:(c + 1) * P],
                                ident, is_transpose=True,
                                start=True, stop=True)
                            nc.vector.tensor_copy(
                                out=ln1T[:, c, q * P:(q + 1) * P], in_=pt)

                    yield from gen_wo_final(
                        nc, t, "sawo", sa_ot, t["sa_wo8"], sa_partial,
                        lambda q, n: xq_sb[:, q, n * 512:(n + 1) * 512],
                        None, "ln1_g", "ln1_b", ln1_out, eps_sb, wop, wops,
                        post_q=post_q)
                    with tc.tile_pool(name="ca_qps", bufs=1,
                                      space="PSUM") as qps:
                        yield from gen_qproj(
                            nc, qt_sb, t["ca_wq8"],
                            lambda tt: ln1T[:, 2 * tt:2 * tt + 2, :], "caq",
                            t1p, qps)

            attention_block(nc, tc, t, name="ca", srcT=t["efT8"],
                            wk=t["ca_wk8"], wv=t["ca_wv8"],
                            qt_fill=ca_qt_fill, qt_rate=0.35,
                            mask_sb=smask_sb, ones_sb=ones_sb,
                            c16_f32=c16_f32,
                            ot_sb=ca_ot, partial_sb=ca_partial,
                            wo_dram=t["ca_wo8"])

            # ===== CA tail: Wo-final -> LN2 -> per-q t2 transposes; then MLP
            with tc.tile_pool(name="mid", bufs=1) as mid:
                ln2_out = mid.tile([P, 4, 1024], F32, tag="ln2o",
                                   name="ln2_out")
                ln2T = mid.tile([P, 8, QS], BF, tag="ln2T", name="ln2T")
                with tc.tile_pool(name="ca_wof", bufs=1) as wop, \
                     tc.tile_pool(name="ca_wofps", bufs=1,
                                  space="PSUM") as wops, \
                     tc.tile_pool(name="t2ps", bufs=1, space="PSUM") as tps2:

                    def post_q2(q):
                        for c in range(8):
                            pt = tps2.tile([P, P], F32, tag="pt", bufs=2,
                                           name=f"t2_pt{c}_{q}")
                            nc.tensor.matmul(
                                pt, ln2_out[:, q, c * P:(c + 1) * P],
                                ident, is_transpose=True,
                                start=True, stop=True)
                            nc.vector.tensor_copy(
                                out=ln2T[:, c, q * P:(q + 1) * P], in_=pt)

                    drain(gen_wo_final(
                        nc, t, "cawo", ca_ot, t["ca_wo8"], ca_partial,
                        lambda q, n: ln1_out[:, q, n * 512:(n + 1) * 512],
                        "ca_bo", "ln2_g", "ln2_b", ln2_out, eps_sb,
                        wop, wops, post_q=post_q2))

                # ================= MLP (bf16) =================
                with tc.tile_pool(name="mlp", bufs=1) as mlp:
                    h2T = mlp.tile([P, 32, QS], BF, tag="h2T", name="h2T")
                    fc1b_sb = mlp.tile([P, 32], F32, tag="f1b",
                                       name="fc1b_sb")
                    nc.sync.dma_start(out=fc1b_sb, in_=t["fc1_b"])
                    fc1_re = t["fc1_w"].rearrange("(c p) n -> p c n", p=P)
                    with tc.tile_pool(name="fc1", bufs=1) as f1p, \
                         tc.tile_pool(name="fc1ps", bufs=1,
                                      space="PSUM") as f1ps:
                        for m in range(32):
                            w_m = f1p.tile([P, 8, P], BF, tag="w",
                                           bufs=3, name=f"fc1w_{m}")
                            nc.sync.dma_start(
                                out=w_m, in_=fc1_re[:, :, m * P:(m + 1) * P])
                            psf = f1ps.tile([P, QS], F32, tag="ps", bufs=2,
                                            name=f"fc1ps_{m}")
                            for d in range(8):
                                nc.tensor.matmul(psf, w_m[:, d, :],
                                                 ln2T[:, d, :],
                                                 start=(d == 0),
                                                 stop=(d == 7))
                            nc.scalar.activation(
                                out=h2T[:, m, :], in_=psf, func=AF.Gelu,
                                bias=fc1b_sb[:, m:m + 1], scale=1.0)
                    fc2_re = t["fc2_w"].rearrange("(c p) n -> p c n", p=P)
                    with tc.tile_pool(name="fc2", bufs=1) as f2p, \
                         tc.tile_pool(name="fc2ps", bufs=1,
                                      space="PSUM") as f2ps:
                        fc2b_sb = f2p.tile([P, 1024], F32, tag="f2b",
                                           name="fc2b_sb")
                        nc.gpsimd.dma_start(out=fc2b_sb,
                                            in_=_dram_bcast(t["fc2_b"], P))
                        ln3g_sb = f2p.tile([P, 1024], F32, tag="l3g",
                                           name="ln3g_sb")
                        nc.gpsimd.dma_start(out=ln3g_sb,
                                            in_=_dram_bcast(t["ln3_g"], P))
                        ln3b_sb = f2p.tile([P, 1024], F32, tag="l3b",
                                           name="ln3b_sb")
                        nc.gpsimd.dma_start(out=ln3b_sb,
                                            in_=_dram_bcast(t["ln3_b"], P))
                        ps2 = [[f2ps.tile([P, 512], F32, tag=f"pr{q}_{n}",
                                          name=f"fc2ps{q}_{n}")
                                for n in range(2)] for q in range(4)]
                        out_re = out.rearrange("(c p) d -> p c d", p=P)
                        for fb in range(8):
                            wb = f2p.tile([P, 4, 1024], BF, tag="wb",
                                          bufs=2, name=f"fc2w_{fb}")
                            nc.sync.dma_start(
                                out=wb,
                                in_=fc2_re[:, fb * 4:(fb + 1) * 4, :])
                            for q in range(4):
                                for n in range(2):
                                    for f in range(4):
                                        ff = fb * 4 + f
                                        nc.tensor.matmul(
                                            ps2[q][n],
                                            h2T[:, ff, q * P:(q + 1) * P],
                                            wb[:, f, n * 512:(n + 1) * 512],
                                            start=(ff == 0),
                                            stop=(ff == 31))
                                if fb == 7:
                                    # q's accumulation done: residual + LN3
                                    # overlap remaining q's matmuls
                                    for n in range(2):
                                        nc.vector.tensor_tensor(
                                            out=ln2_out[:, q, n * 512:(n + 1) * 512],
                                            in0=ps2[q][n],
                                            in1=ln2_out[:, q, n * 512:(n + 1) * 512],
                                            op=ALU.add)
                                    nc.vector.tensor_tensor(
                                        out=ln2_out[:, q, :],
                                        in0=ln2_out[:, q, :],
                                        in1=fc2b_sb, op=ALU.add)
                                    o_sb = f2p.tile([P, 1024], F32, tag="osb",
                                                    bufs=2, name=f"out_sb{q}")
                                    _layernorm_q(nc, f2p, f"ln3_{q}", ln2_out,
                                                 q, ln3g_sb, ln3b_sb, o_sb,
                                                 eps_sb)
                                    nc.sync.dma_start(out=out_re[:, q, :],
                                                      in_=o_sb)
    nc.compile()
    return nc


_NC_CACHE = None
_RUNNER_CACHE = None


def _get_nc():
    global _NC_CACHE
    if _NC_CACHE is None:
        _NC_CACHE = build_nc()
    return _NC_CACHE


def _build_runner(nc, n_cores):
    """Cached jitted SPMD executor (same machinery run_bass_kernel_spmd uses
    under axon, but reusable across kernel() calls)."""
    import jax
    from jax.sharding import Mesh, PartitionSpec
    from jax.experimental.shard_map import shard_map
    from concourse import bass2jax

    bass2jax.install_neuronx_cc_hook()
    part_name = nc.partition_id_tensor.name if nc.partition_id_tensor else None
    in_names, out_names, out_avals, zero_shapes = [], [], [], []
    for alloc in nc.m.functions[0].allocations:
        if not isinstance(alloc, mybir.MemoryLocationSet):
            continue
        name = alloc.memorylocations[0].name
        if alloc.kind == "ExternalInput":
            if name != part_name:
                in_names.append(name)
        elif alloc.kind == "ExternalOutput":
            out_names.append(name)
            shape = tuple(alloc.tensor_shape)
            dtype = mybir.dt.np(alloc.dtype)
            out_avals.append(jax.core.ShapedArray(shape, dtype))
            zero_shapes.append((shape, dtype))
    n_params = len(in_names)
    all_names = list(in_names) + list(out_names)
    if part_name is not None:
        all_names.append(part_name)

    def _body(*args):
        operands = list(args)
        if part_name is not None:
            operands.append(bass2jax.partition_id_tensor())
        outs = bass2jax._bass_exec_p.bind(
            *operands, out_avals=tuple(out_avals), in_names=tuple(all_names),
            out_names=tuple(out_names), lowering_input_output_aliases=(),
            sim_require_finite=True, sim_require_nnan=True, nc=nc)
        return tuple(outs)

    devices = jax.devices()[:n_cores]
    mesh = Mesh(np.asarray(devices), ("core",))
    in_specs = (PartitionSpec("core"),) * (n_params + len(out_names))
    out_specs = (PartitionSpec("core"),) * len(out_names)
    fn = jax.jit(shard_map(_body, mesh=mesh, in_specs=in_specs,
                           out_specs=out_specs, check_rep=False),
                 keep_unused=True)
    sharding = jax.sharding.NamedSharding(mesh, PartitionSpec("core"))
    return fn, in_names, out_names, out_avals, zero_shapes, sharding


def _run_fast(in_maps):
    """Execute via the cached jitted runner. Returns per-core out dicts."""
    global _RUNNER_CACHE
    import jax
    nc = _get_nc()
    if _RUNNER_CACHE is None:
        _RUNNER_CACHE = _build_runner(nc, NCORES)
    fn, in_names, out_names, out_avals, zero_shapes, sharding = _RUNNER_CACHE
    concat_in = [
        np.concatenate([np.asarray(in_maps[c][nm]) for c in range(NCORES)],
                       axis=0)
        for nm in in_names
    ]
    concat_zeros = [np.zeros((NCORES * s[0], *s[1:]), d)
                    for (s, d) in zero_shapes]
    dev_in = [jax.device_put(a, sharding) for a in concat_in]
    dev_z = [jax.device_put(a, sharding) for a in concat_zeros]
    outs = fn(*dev_in, *dev_z)
    jax.block_until_ready(outs)
    return [
        {nm: np.asarray(outs[i]).reshape(NCORES, *out_avals[i].shape)[c]
         for i, nm in enumerate(out_names)}
        for c in range(NCORES)
    ]


def make_in_maps(inputs):
    f32c = lambda a: np.ascontiguousarray(np.asarray(a), dtype=np.float32)
    bfc = lambda a: np.ascontiguousarray(np.asarray(a, dtype=np.float32)
                                         .astype(ml_dtypes.bfloat16))
    f8c = lambda a, s=1.0: np.ascontiguousarray(
        (np.asarray(a, dtype=np.float32) * np.float32(s))
        .astype(ml_dtypes.float8_e4m3))
    x = f32c(inputs["x"]); enc = f32c(inputs["enc_output"])
    tgt = np.asarray(inputs["tgt_mask"]); src = np.asarray(inputs["src_mask"])

    shared = {
        "sa_wq8": f8c(inputs["sa_wq"], WSC),
        "ca_wq8": f8c(inputs["ca_wq"], WSC),
        "sa_wk8": f8c(inputs["sa_wk"], WSC), "sa_wv8": f8c(inputs["sa_wv"], WSC),
        "ca_wk8": f8c(inputs["ca_wk"], WSC), "ca_wv8": f8c(inputs["ca_wv"], WSC),
        "sa_wo8": f8c(inputs["sa_wo"], WSC), "ca_wo8": f8c(inputs["ca_wo"], WSC),
        "fc1_w": bfc(inputs["fc1_w"]), "fc2_w": bfc(inputs["fc2_w"]),
        "fc1_b": np.ascontiguousarray(f32c(inputs["fc1_b"]).reshape(32, P).T),
        "fc2_b": f32c(inputs["fc2_b"]), "ca_bo": f32c(inputs["ca_bo"]),
        "ln1_g": f32c(inputs["ln1_g"]), "ln1_b": f32c(inputs["ln1_b"]),
        "ln2_g": f32c(inputs["ln2_g"]), "ln2_b": f32c(inputs["ln2_b"]),
        "ln3_g": f32c(inputs["ln3_g"]), "ln3_b": f32c(inputs["ln3_b"]),
    }
    shared = {k: np.ascontiguousarray(v) for k, v in shared.items()}

    in_maps = []
    for c in range(NCORES):
        b, qi = c // 4, c % 4
        q0 = qi * QS
        xb = x[b]; eb = enc[b]
        tm = np.where(tgt[b], np.float32(NEG),
                      np.float32(SHIFT)).astype(np.float32)
        sm = np.where(src[b], np.float32(NEG),
                      np.float32(SHIFT)).astype(np.float32)
        m = dict(shared)
        m["xqT8"] = f8c(xb[q0:q0 + QS].T)
        m["xq"] = np.ascontiguousarray(xb[q0:q0 + QS] + f32c(inputs["sa_bo"]))
        m["xfT8"] = f8c(xb.T)
        m["efT8"] = f8c(eb.T)
        m["tmask"] = np.ascontiguousarray(tm.reshape(16, P).T)
        m["smask"] = np.ascontiguousarray(sm.reshape(16, P).T)
        in_maps.append(m)
    return in_maps


def kernel(**inputs):
    in_maps = make_in_maps(inputs)
    try:
        results = _run_fast(in_maps)
    except Exception:
        nc = _get_nc()
        results = run_bass_kernel_spmd(nc, in_maps,
                                       core_ids=list(range(NCORES))).results
    out = np.empty((B, S, D), np.float32)
    for c in range(NCORES):
        b, qi = c // 4, c % 4
        out[b, qi * QS:(qi + 1) * QS] = results[c]["out"]
    return out
